# revision 36
# baseline (speedup 1.0000x reference)
"""Multi-head attention (B=2, T=2048, D=1024, H=16, dh=64) on 8 TRN2 NeuronCores.

Sharding: batch x head-group. Core i handles batch b=i//4 and heads
[4g, 4g+4) with g=i%4. Per core:
  - qk^T projection in transposed layout [feat, tok], accumulated c-outer
    in two PSUM waves so matmuls start as soon as each input chunk's DMA
    lands (no separate warmup needed, no PE-idle load phase)
  - v projection [tok, feat] with a ones-column appended per head, so the
    PV matmul emits the softmax denominator on PSUM partition 64 for free
  - attention over 4 blocks of 512 q tokens; scores contract K=64 directly
    at partition offset 64*(h%2) (no zero-padded k^T copies); exp split
    between ScalarE (exact) and VectorE (Schraudolph int16 bitcast)
  - denominator broadcast via a K=1 PE matmul (ones row), reciprocal +
    multiply on VectorE/GpSimd -> normalized head outputs
  - per 512-token block: partial output projection over this core's 4
    heads only, then a 4-core ReduceScatter sums partials across the
    batch group; comm overlaps the next block's compute
Host assembles the [2, 2048, 1024] float32 result from bf16 shards.
"""
from contextlib import ExitStack

import numpy as np
import ml_dtypes

import concourse.bass as bass
import concourse.mybir as mybir
import concourse.tile as tile
from concourse import bacc
from concourse.bass_utils import run_bass_kernel_spmd

BF16 = mybir.dt.bfloat16
F32 = mybir.dt.float32
I16 = mybir.dt.int16

B, T, D = 2, 2048, 1024
D_HEAD = 64
N_CORES = 8
H_LOC = 4            # heads per core
E_QK = 512           # q+k features per core
E_V = 256            # v features per core
N_DC = D // 128      # 8 contraction chunks for projections
N_KC = T // 128      # 16 k-chunks
QT = 512             # q tokens per attention block
N_QB = T // QT       # 4 blocks
VW = 2 * D_HEAD      # v columns per head: [64 v | 64 ones]

# Schraudolph exp in bf16 bit domain: bf16bits(exp(x)) ~ int16(A*x + B)
EXP_A = float(128.0 / np.log(2.0))
EXP_B = float(127.0 * 128.0 - 5.5)

REPLICA_GROUPS = [[0, 1, 2, 3], [4, 5, 6, 7]]


def build_nc():
    nc = bacc.Bacc("TRN2", target_bir_lowering=False, debug=False,
                   num_devices=N_CORES)

    xt_ext = nc.dram_tensor("xt", [D, T], BF16, kind="ExternalInput")
    wqk_ext = nc.dram_tensor("wqk", [D, E_QK], BF16, kind="ExternalInput")
    wv_ext = nc.dram_tensor("wv", [D, E_V], BF16, kind="ExternalInput")
    wo_ext = nc.dram_tensor("wo", [E_V, D], BF16, kind="ExternalInput")
    out_ext = nc.dram_tensor("out", [N_QB * 128, D], BF16,
                             kind="ExternalOutput")

    with tile.TileContext(nc) as tc:
        with (
            tc.tile_pool(name="persist", bufs=1) as persist,
            tc.tile_pool(name="work", bufs=4) as work,
            tc.tile_pool(name="dram", bufs=1, space="DRAM") as dram,
        ):
            # ---- inputs: interleave wqk/xt chunks so QK proj can start early
            xt_sb = persist.tile([128, N_DC, T], BF16)
            wqk_sb = persist.tile([128, N_DC, E_QK], BF16)
            wv_sb = persist.tile([128, N_DC, E_V], BF16)
            wo_sb = persist.tile([128, 2, D], BF16)
            for c in range(N_DC):
                nc.sync.dma_start(out=wqk_sb[:, c, :],
                                  in_=wqk_ext[128 * c:128 * (c + 1), :])
                for t2 in range(2):
                    nc.sync.dma_start(
                        out=xt_sb[:, c, 1024 * t2:1024 * (t2 + 1)],
                        in_=xt_ext[128 * c:128 * (c + 1),
                                   1024 * t2:1024 * (t2 + 1)])
            for c in range(N_DC):
                nc.sync.dma_start(out=wv_sb[:, c, :],
                                  in_=wv_ext[128 * c:128 * (c + 1), :])
            for j in range(2):
                nc.gpsimd.dma_start(out=wo_sb[:, j, :],
                                    in_=wo_ext[128 * j:128 * (j + 1), :])

            # ones for the v ones-columns (PV then emits the softmax
            # denominator replicated on output partitions 64-127 for free)
            vext_sb = persist.tile([128, N_KC, H_LOC * VW], BF16)
            nc.gpsimd.memset(vext_sb[:], 1.0)
            ones_sb = persist.tile([128, D_HEAD], BF16)
            nc.vector.memset(ones_sb[:], 1.0)

            # ---- qk^T projection: [E_QK, T], c-outer in two PSUM waves ----
            proj_stack = ExitStack()
            ps_proj = proj_stack.enter_context(
                tc.tile_pool(name="ps_proj", bufs=1, space="PSUM"))
            qkt_sb = persist.tile([128, E_QK // 128, T], BF16)
            for w in range(2):
                tiles = {}
                for t in range(4):
                    for eo in range(2):
                        tiles[(t, eo)] = ps_proj.tile(
                            [128, 512], F32, tag=f"qk_{t}_{eo}",
                            name=f"qk_{w}_{t}_{eo}")
                for c in range(N_DC):
                    for t in range(4):
                        for eo in range(2):
                            e = 2 * w + eo
                            nc.tensor.matmul(
                                tiles[(t, eo)][:],
                                wqk_sb[:, c, 128 * e:128 * (e + 1)],
                                xt_sb[:, c, 512 * t:512 * (t + 1)],
                                start=(c == 0), stop=(c == N_DC - 1),
                            )
                for t in range(4):
                    for eo in range(2):
                        e = 2 * w + eo
                        eng = nc.vector if eo == 0 else nc.scalar
                        dst = qkt_sb[:, e, 512 * t:512 * (t + 1)]
                        if eo == 0:
                            eng.tensor_copy(dst, tiles[(t, eo)][:])
                        else:
                            eng.copy(dst, tiles[(t, eo)][:])

            proj_stack.close()

            # ---- v projection (+ ones column): vext [tok, H_LOC*65] -------
            vproj_stack = ExitStack()
            ps_vp = vproj_stack.enter_context(
                tc.tile_pool(name="ps_vp", bufs=1, space="PSUM"))
            for tk in range(N_KC):
                ps = ps_vp.tile([128, E_V], F32, tag="v", bufs=2,
                                name=f"ps_v_{tk}")
                for c in range(N_DC):
                    nc.tensor.matmul(
                        ps[:],
                        xt_sb[:, c, 128 * tk:128 * (tk + 1)],
                        wv_sb[:, c, :],
                        start=(c == 0), stop=(c == N_DC - 1),
                    )
                dst = vext_sb[:, tk, :].rearrange(
                    "p (h c) -> p h c", h=H_LOC)[:, :, 0:D_HEAD]
                src = ps[:].rearrange("p (h c) -> p h c", c=D_HEAD)
                if tk % 2 == 0:
                    nc.vector.tensor_copy(dst, src)
                else:
                    nc.scalar.copy(dst, src)
            vproj_stack.close()

            # ---- attention + partial out-proj + ReduceScatter per block ---
            attn_stack = ExitStack()
            ps_s = attn_stack.enter_context(
                tc.tile_pool(name="ps_s", bufs=2, space="PSUM"))
            ps_o = attn_stack.enter_context(
                tc.tile_pool(name="ps_o", bufs=2, space="PSUM"))
            ps_f = attn_stack.enter_context(
                tc.tile_pool(name="ps_f", bufs=2, space="PSUM"))

            rs_in = [dram.tile([QT, D], BF16, name=f"rs_in_{qb}")
                     for qb in range(N_QB)]
            rs_out = [dram.tile([128, D], BF16, name=f"rs_out_{qb}")
                      for qb in range(N_QB)]

            # out-proj for block qb is emitted as units interleaved into
            # block qb+1's first kc loop, so its PSUM-evacuation copies
            # don't pile onto ACT/DVE at the block boundary (where they
            # starve the exps that the PE's score pipeline waits on)
            def make_outproj(qb, attnp_blk):
                units = []
                for ts in range(QT // 128):
                    def emit(ts=ts):
                        out_sb = work.tile([128, D], BF16, tag="out_sb",
                                           bufs=4, name=f"outsb_{qb}_{ts}")
                        for n in range(2):
                            ps = ps_f.tile([128, 512], F32, tag="f",
                                           name=f"psf_{qb}_{ts}_{n}")
                            for pair in range(2):
                                nc.tensor.matmul(
                                    ps[:],
                                    attnp_blk[pair][:, 128 * ts:128 * (ts + 1)],
                                    wo_sb[:, pair, 512 * n:512 * (n + 1)],
                                    start=(pair == 0), stop=(pair == 1),
                                )
                            if (ts + n) % 2 == 0:
                                nc.vector.tensor_copy(
                                    out_sb[:, 512 * n:512 * (n + 1)], ps[:])
                            else:
                                nc.scalar.copy(
                                    out_sb[:, 512 * n:512 * (n + 1)], ps[:])
                        nc.sync.dma_start(
                            out=rs_in[qb][128 * ts:128 * (ts + 1), :],
                            in_=out_sb[:])
                    units.append(emit)

                def finish():
                    nc.gpsimd.collective_compute(
                        "ReduceScatter",
                        mybir.AluOpType.add,
                        replica_groups=REPLICA_GROUPS,
                        ins=[rs_in[qb][:].opt()],
                        outs=[rs_out[qb][:].opt()],
                    )
                units.append(finish)
                return units

            for qb in range(N_QB):
                q0 = QT * qb
                attnp = {}
                for pair in range(2):
                    heads = (2 * pair, 2 * pair + 1)
                    attnp[pair] = work.tile([128, QT], BF16, tag=f"at{pair}",
                                            bufs=3, name=f"attnp_{pair}_{qb}")
                    po = {}
                    for h in heads:
                        po[h] = ps_o.tile([128, QT], F32, tag="po",
                                          name=f"po_{h}_{qb}")

                    # software pipeline: scores(kc) + one merged exp(kc) for
                    # both heads (engines alternate per kc) + PV(kc-2), so
                    # exp has two iterations of latency budget
                    def pv_mm(kc, p_sb):
                        for h in heads:
                            nc.tensor.matmul(
                                po[h][0:128, :],
                                vext_sb[:, kc, VW * h:VW * (h + 1)],
                                p_sb[:, 512 * (h % 2):512 * (h % 2) + 512],
                                start=(kc == 0), stop=(kc == N_KC - 1),
                            )

                    p_hist = []
                    for kc in range(N_KC):
                        s_ps = ps_s.tile([128, 2 * QT], F32, tag="s",
                                         name=f"s_{pair}_{qb}_{kc}")
                        for h in heads:
                            m = 64 * (h % 2)
                            nc.tensor.matmul(
                                s_ps[:, 512 * (h % 2):512 * (h % 2) + 512],
                                qkt_sb[m:m + 64, 2 + h // 2,
                                       128 * kc:128 * (kc + 1)],
                                qkt_sb[m:m + 64, h // 2, q0:q0 + QT],
                                start=True, stop=True,
                            )
                        p_sb = work.tile([128, 2 * QT], BF16,
                                         tag=f"p{kc % 2}", bufs=4,
                                         name=f"p_{pair}_{qb}_{kc}")
                        if kc % 2 == 0:
                            nc.scalar.activation(
                                p_sb[:], s_ps[:],
                                mybir.ActivationFunctionType.Exp)
                        else:
                            nc.vector.tensor_scalar(
                                p_sb[:].bitcast(I16), s_ps[:],
                                EXP_A, EXP_B,
                                mybir.AluOpType.mult, mybir.AluOpType.add,
                            )
                        p_hist.append(p_sb)
                        if kc >= 2:
                            pv_mm(kc - 2, p_hist[kc - 2])
                    pv_mm(N_KC - 2, p_hist[N_KC - 2])
                    pv_mm(N_KC - 1, p_hist[N_KC - 1])
                    # normalize: evacuate pv rows + one den row, broadcast
                    # the denominator across partitions with a K=1 matmul
                    # (engines cannot mix start partitions; the PE can)
                    for h in heads:
                        po_sb = work.tile([128, QT], BF16, tag="po_sb", bufs=4,
                                          name=f"posb_{h}_{qb}")
                        if h % 2 == 0:
                            nc.scalar.copy(po_sb[0:65, :], po[h][0:65, :])
                        else:
                            nc.vector.tensor_copy(po_sb[0:65, :],
                                                  po[h][0:65, :])
                        den_ps = ps_f.tile([128, QT], F32, tag="f",
                                           name=f"den_{h}_{qb}")
                        nc.tensor.matmul(
                            den_ps[0:64, :],
                            ones_sb[64:65, :],
                            po_sb[64:65, :],
                            start=True, stop=True,
                        )
                        rc = work.tile([64, QT], F32, tag="rc", bufs=4,
                                       name=f"rc_{h}_{qb}")
                        nc.vector.reciprocal_approx_fast(rc[:], den_ps[0:64, :])
                        if h % 2 == 0:
                            nc.vector.tensor_mul(attnp[pair][0:64, :],
                                                 po_sb[0:64, :], rc[:])
                        else:
                            atmp = work.tile([64, QT], BF16, tag="atmp",
                                             bufs=2, name=f"atmp_{h}_{qb}")
                            nc.vector.tensor_mul(atmp[:], po_sb[0:64, :], rc[:])
                            nc.sync.dma_start(out=attnp[pair][64:128, :],
                                              in_=atmp[:])
                for unit in make_outproj(qb, attnp):
                    unit()
            # out DMAs after the last RS trigger: blocks 0-2 fire instantly
            # (their RS long done); only the last blocks on the final RS.
            # Keeping them off the earlier queue positions means an
            # unfinished RS never stalls work queued behind its out DMA.
            for qb in range(N_QB):
                nc.gpsimd.dma_start(out=out_ext[128 * qb:128 * (qb + 1), :],
                                    in_=rs_out[qb][:])
            attn_stack.close()

    nc.compile()
    return nc


_NC = None


def _get_nc():
    global _NC
    if _NC is None:
        _NC = build_nc()
    return _NC


def kernel(x, Wqkv, Wo):
    bf16 = ml_dtypes.bfloat16
    s = np.float32(1.0 / np.sqrt(D_HEAD))

    xt = [np.ascontiguousarray(np.asarray(x)[b].T).astype(bf16)
          for b in range(B)]
    Wqkv = np.asarray(Wqkv)
    Wo = np.asarray(Wo)

    in_maps = []
    for i in range(N_CORES):
        b, g = divmod(i, 4)
        wq = Wqkv[256 * g:256 * (g + 1)] * s
        wk = Wqkv[D + 256 * g:D + 256 * (g + 1)]
        wqk = np.ascontiguousarray(
            np.concatenate([wq, wk], axis=0).T).astype(bf16)
        wv = np.ascontiguousarray(
            Wqkv[2 * D + 256 * g:2 * D + 256 * (g + 1)].T).astype(bf16)
        wo = np.ascontiguousarray(Wo[:, 256 * g:256 * (g + 1)].T).astype(bf16)
        in_maps.append({"xt": xt[b], "wqk": wqk, "wv": wv, "wo": wo})

    nc = _get_nc()
    res = run_bass_kernel_spmd(nc, in_maps, core_ids=list(range(N_CORES)))

    out = np.empty((B, T, D), dtype=np.float32)
    for i in range(N_CORES):
        b, r = divmod(i, 4)
        o = res.results[i]["out"].astype(np.float32)
        for qb in range(N_QB):
            out[b, QT * qb + 128 * r:QT * qb + 128 * (r + 1), :] = \
                o[128 * qb:128 * (qb + 1)]
    return out


# revision 37
# speedup vs baseline: 1.1446x; 1.1446x over previous
"""Multi-head attention (B=2, T=2048, D=1024, H=16, dh=64) on 8 TRN2 NeuronCores.

Sharding: batch x head-group. Core i handles batch b=i//4 and heads
[4g, 4g+4) with g=i%4. Per core:
  - qk^T projection in transposed layout [feat, tok], accumulated c-outer
    in two PSUM waves so matmuls start as soon as each input chunk's DMA
    lands (no separate warmup needed, no PE-idle load phase)
  - v projection [tok, feat] with a ones-column appended per head, so the
    PV matmul emits the softmax denominator on PSUM partition 64 for free
  - attention over 4 blocks of 512 q tokens; scores contract K=64 directly
    at partition offset 64*(h%2) (no zero-padded k^T copies); exp split
    between ScalarE (exact) and VectorE (Schraudolph int16 bitcast)
  - denominator broadcast via a K=1 PE matmul (ones row), reciprocal +
    multiply on VectorE/GpSimd -> normalized head outputs
  - per 512-token block: partial output projection over this core's 4
    heads only, then a 4-core ReduceScatter sums partials across the
    batch group; comm overlaps the next block's compute
Host assembles the [2, 2048, 1024] float32 result from bf16 shards.
"""
from contextlib import ExitStack

import numpy as np
import ml_dtypes

import concourse.bass as bass
import concourse.mybir as mybir
import concourse.tile as tile
from concourse import bacc
from concourse.bass_utils import run_bass_kernel_spmd

BF16 = mybir.dt.bfloat16
F32 = mybir.dt.float32
I16 = mybir.dt.int16

B, T, D = 2, 2048, 1024
D_HEAD = 64
N_CORES = 8
H_LOC = 4            # heads per core
E_QK = 512           # q+k features per core
E_V = 256            # v features per core
N_DC = D // 128      # 8 contraction chunks for projections
N_KC = T // 128      # 16 k-chunks
QT = 512             # q tokens per attention block
N_QB = T // QT       # 4 blocks
VW = 2 * D_HEAD      # v columns per head: [64 v | 64 ones]

# Schraudolph exp in bf16 bit domain: bf16bits(exp(x)) ~ int16(A*x + B)
EXP_A = float(128.0 / np.log(2.0))
EXP_B = float(127.0 * 128.0 - 5.5)

REPLICA_GROUPS = [[0, 1, 2, 3], [4, 5, 6, 7]]


def build_nc():
    nc = bacc.Bacc("TRN2", target_bir_lowering=False, debug=False,
                   num_devices=N_CORES)

    xt_ext = nc.dram_tensor("xt", [D, T], BF16, kind="ExternalInput")
    wqk_ext = nc.dram_tensor("wqk", [D, E_QK], BF16, kind="ExternalInput")
    wv_ext = nc.dram_tensor("wv", [D, E_V], BF16, kind="ExternalInput")
    wo_ext = nc.dram_tensor("wo", [E_V, D], BF16, kind="ExternalInput")
    out_ext = nc.dram_tensor("out", [N_QB * 128, D], BF16,
                             kind="ExternalOutput")

    with tile.TileContext(nc) as tc:
        with (
            tc.tile_pool(name="persist", bufs=1) as persist,
            tc.tile_pool(name="work", bufs=4) as work,
            tc.tile_pool(name="dram", bufs=1, space="DRAM") as dram,
        ):
            # ---- inputs: interleave wqk/xt chunks so QK proj can start early
            xt_sb = persist.tile([128, N_DC, T], BF16)
            wqk_sb = persist.tile([128, N_DC, E_QK], BF16)
            wv_sb = persist.tile([128, N_DC, E_V], BF16)
            wo_sb = persist.tile([128, 2, D], BF16)
            for c in range(N_DC):
                nc.sync.dma_start(out=wqk_sb[:, c, :],
                                  in_=wqk_ext[128 * c:128 * (c + 1), :])
                for t2 in range(2):
                    nc.sync.dma_start(
                        out=xt_sb[:, c, 1024 * t2:1024 * (t2 + 1)],
                        in_=xt_ext[128 * c:128 * (c + 1),
                                   1024 * t2:1024 * (t2 + 1)])
            for c in range(N_DC):
                nc.sync.dma_start(out=wv_sb[:, c, :],
                                  in_=wv_ext[128 * c:128 * (c + 1), :])
            for j in range(2):
                nc.gpsimd.dma_start(out=wo_sb[:, j, :],
                                    in_=wo_ext[128 * j:128 * (j + 1), :])

            # ones for the v ones-columns (PV then emits the softmax
            # denominator replicated on output partitions 64-127 for free)
            vext_sb = persist.tile([128, N_KC, H_LOC * VW], BF16)
            nc.gpsimd.memset(vext_sb[:], 1.0)
            ones_sb = persist.tile([128, D_HEAD], BF16)
            nc.vector.memset(ones_sb[:], 1.0)

            # ---- qk^T projection: [E_QK, T], c-outer in two PSUM waves ----
            proj_stack = ExitStack()
            ps_proj = proj_stack.enter_context(
                tc.tile_pool(name="ps_proj", bufs=1, space="PSUM"))
            qkt_sb = persist.tile([128, E_QK // 128, T], BF16)
            for w in range(2):
                tiles = {}
                for t in range(4):
                    for eo in range(2):
                        tiles[(t, eo)] = ps_proj.tile(
                            [128, 512], F32, tag=f"qk_{t}_{eo}",
                            name=f"qk_{w}_{t}_{eo}")
                for c in range(N_DC):
                    for t in range(4):
                        for eo in range(2):
                            e = 2 * w + eo
                            nc.tensor.matmul(
                                tiles[(t, eo)][:],
                                wqk_sb[:, c, 128 * e:128 * (e + 1)],
                                xt_sb[:, c, 512 * t:512 * (t + 1)],
                                start=(c == 0), stop=(c == N_DC - 1),
                            )
                for t in range(4):
                    for eo in range(2):
                        e = 2 * w + eo
                        eng = nc.vector if eo == 0 else nc.scalar
                        dst = qkt_sb[:, e, 512 * t:512 * (t + 1)]
                        if eo == 0:
                            eng.tensor_copy(dst, tiles[(t, eo)][:])
                        else:
                            eng.copy(dst, tiles[(t, eo)][:])

            proj_stack.close()

            # ---- v projection (+ ones column): vext [tok, H_LOC*65] -------
            vproj_stack = ExitStack()
            ps_vp = vproj_stack.enter_context(
                tc.tile_pool(name="ps_vp", bufs=1, space="PSUM"))
            for tk in range(N_KC):
                ps = ps_vp.tile([128, E_V], F32, tag="v", bufs=2,
                                name=f"ps_v_{tk}")
                for c in range(N_DC):
                    nc.tensor.matmul(
                        ps[:],
                        xt_sb[:, c, 128 * tk:128 * (tk + 1)],
                        wv_sb[:, c, :],
                        start=(c == 0), stop=(c == N_DC - 1),
                    )
                dst = vext_sb[:, tk, :].rearrange(
                    "p (h c) -> p h c", h=H_LOC)[:, :, 0:D_HEAD]
                src = ps[:].rearrange("p (h c) -> p h c", c=D_HEAD)
                if tk % 2 == 0:
                    nc.vector.tensor_copy(dst, src)
                else:
                    nc.scalar.copy(dst, src)
            vproj_stack.close()

            # ---- attention + partial out-proj + ReduceScatter per block ---
            attn_stack = ExitStack()
            ps_s = attn_stack.enter_context(
                tc.tile_pool(name="ps_s", bufs=2, space="PSUM"))
            ps_o = attn_stack.enter_context(
                tc.tile_pool(name="ps_o", bufs=2, space="PSUM"))
            ps_f = attn_stack.enter_context(
                tc.tile_pool(name="ps_f", bufs=2, space="PSUM"))

            rs_in = [dram.tile([QT, D], BF16, name=f"rs_in_{qb}")
                     for qb in range(N_QB)]
            rs_out = [dram.tile([128, D], BF16, name=f"rs_out_{qb}")
                      for qb in range(N_QB)]

            # out-proj for block qb is emitted as units interleaved into
            # block qb+1's first kc loop, so its PSUM-evacuation copies
            # don't pile onto ACT/DVE at the block boundary (where they
            # starve the exps that the PE's score pipeline waits on)
            def make_outproj(qb, attnp_blk):
                units = []
                for ts in range(QT // 128):
                    def emit(ts=ts):
                        out_sb = work.tile([128, D], BF16, tag="out_sb",
                                           bufs=3, name=f"outsb_{qb}_{ts}")
                        for n in range(2):
                            ps = ps_f.tile([128, 512], F32, tag="f",
                                           name=f"psf_{qb}_{ts}_{n}")
                            for pair in range(2):
                                nc.tensor.matmul(
                                    ps[:],
                                    attnp_blk[pair][:, 128 * ts:128 * (ts + 1)],
                                    wo_sb[:, pair, 512 * n:512 * (n + 1)],
                                    start=(pair == 0), stop=(pair == 1),
                                )
                            if (ts + n) % 2 == 0:
                                nc.vector.tensor_copy(
                                    out_sb[:, 512 * n:512 * (n + 1)], ps[:])
                            else:
                                nc.scalar.copy(
                                    out_sb[:, 512 * n:512 * (n + 1)], ps[:])
                        nc.sync.dma_start(
                            out=rs_in[qb][128 * ts:128 * (ts + 1), :],
                            in_=out_sb[:])
                    units.append(emit)

                def finish():
                    nc.gpsimd.collective_compute(
                        "ReduceScatter",
                        mybir.AluOpType.add,
                        replica_groups=REPLICA_GROUPS,
                        ins=[rs_in[qb][:].opt()],
                        outs=[rs_out[qb][:].opt()],
                    )
                units.append(finish)
                return units

            for qb in range(N_QB):
                q0 = QT * qb
                attnp = {}
                for pair in range(2):
                    heads = (2 * pair, 2 * pair + 1)
                    attnp[pair] = work.tile([128, QT], BF16, tag=f"at{pair}",
                                            bufs=2, name=f"attnp_{pair}_{qb}")
                    po = {}
                    for h in heads:
                        po[h] = ps_o.tile([128, QT], F32, tag="po",
                                          name=f"po_{h}_{qb}")

                    # software pipeline: scores(kc) + one merged exp(kc) for
                    # both heads (engines alternate per kc) + PV(kc-2), so
                    # exp has two iterations of latency budget
                    def pv_mm(kc, p_sb):
                        for h in heads:
                            nc.tensor.matmul(
                                po[h][0:128, :],
                                vext_sb[:, kc, VW * h:VW * (h + 1)],
                                p_sb[:, 512 * (h % 2):512 * (h % 2) + 512],
                                start=(kc == 0), stop=(kc == N_KC - 1),
                            )

                    p_hist = []
                    for kc in range(N_KC):
                        s_ps = ps_s.tile([128, 2 * QT], F32, tag="s",
                                         name=f"s_{pair}_{qb}_{kc}")
                        for h in heads:
                            m = 64 * (h % 2)
                            nc.tensor.matmul(
                                s_ps[:, 512 * (h % 2):512 * (h % 2) + 512],
                                qkt_sb[m:m + 64, 2 + h // 2,
                                       128 * kc:128 * (kc + 1)],
                                qkt_sb[m:m + 64, h // 2, q0:q0 + QT],
                                start=True, stop=True,
                            )
                        p_sb = work.tile([128, 2 * QT], BF16,
                                         tag=f"p{kc % 2}", bufs=3,
                                         name=f"p_{pair}_{qb}_{kc}")
                        if kc % 2 == 0:
                            nc.scalar.activation(
                                p_sb[:], s_ps[:],
                                mybir.ActivationFunctionType.Exp)
                        else:
                            nc.vector.tensor_scalar(
                                p_sb[:].bitcast(I16), s_ps[:],
                                EXP_A, EXP_B,
                                mybir.AluOpType.mult, mybir.AluOpType.add,
                            )
                        p_hist.append(p_sb)
                        if kc >= 2:
                            pv_mm(kc - 2, p_hist[kc - 2])
                    pv_mm(N_KC - 2, p_hist[N_KC - 2])
                    pv_mm(N_KC - 1, p_hist[N_KC - 1])
                    # normalize: evacuate pv rows + one den row, broadcast
                    # the denominator across partitions with a K=1 matmul
                    # (engines cannot mix start partitions; the PE can)
                    for h in heads:
                        po_sb = work.tile([128, QT], BF16, tag="po_sb", bufs=4,
                                          name=f"posb_{h}_{qb}")
                        if h % 2 == 0:
                            nc.scalar.copy(po_sb[0:65, :], po[h][0:65, :])
                        else:
                            nc.vector.tensor_copy(po_sb[0:65, :],
                                                  po[h][0:65, :])
                        den_ps = ps_f.tile([128, QT], F32, tag="f",
                                           name=f"den_{h}_{qb}")
                        nc.tensor.matmul(
                            den_ps[0:64, :],
                            ones_sb[64:65, :],
                            po_sb[64:65, :],
                            start=True, stop=True,
                        )
                        rc = work.tile([64, QT], F32, tag="rc", bufs=4,
                                       name=f"rc_{h}_{qb}")
                        nc.vector.reciprocal_approx_fast(rc[:], den_ps[0:64, :])
                        if h % 2 == 0:
                            nc.vector.tensor_mul(attnp[pair][0:64, :],
                                                 po_sb[0:64, :], rc[:])
                        else:
                            atmp = work.tile([64, QT], BF16, tag="atmp",
                                             bufs=2, name=f"atmp_{h}_{qb}")
                            nc.vector.tensor_mul(atmp[:], po_sb[0:64, :], rc[:])
                            nc.sync.dma_start(out=attnp[pair][64:128, :],
                                              in_=atmp[:])
                for unit in make_outproj(qb, attnp):
                    unit()
            # out DMAs after the last RS trigger: blocks 0-2 fire instantly
            # (their RS long done); only the last blocks on the final RS.
            # Keeping them off the earlier queue positions means an
            # unfinished RS never stalls work queued behind its out DMA.
            for qb in range(N_QB):
                nc.gpsimd.dma_start(out=out_ext[128 * qb:128 * (qb + 1), :],
                                    in_=rs_out[qb][:])
            attn_stack.close()

    nc.compile()
    return nc


_NC = None


def _get_nc():
    global _NC
    if _NC is None:
        _NC = build_nc()
    return _NC


def kernel(x, Wqkv, Wo):
    bf16 = ml_dtypes.bfloat16
    s = np.float32(1.0 / np.sqrt(D_HEAD))

    xt = [np.ascontiguousarray(np.asarray(x)[b].T).astype(bf16)
          for b in range(B)]
    Wqkv = np.asarray(Wqkv)
    Wo = np.asarray(Wo)

    in_maps = []
    for i in range(N_CORES):
        b, g = divmod(i, 4)
        wq = Wqkv[256 * g:256 * (g + 1)] * s
        wk = Wqkv[D + 256 * g:D + 256 * (g + 1)]
        wqk = np.ascontiguousarray(
            np.concatenate([wq, wk], axis=0).T).astype(bf16)
        wv = np.ascontiguousarray(
            Wqkv[2 * D + 256 * g:2 * D + 256 * (g + 1)].T).astype(bf16)
        wo = np.ascontiguousarray(Wo[:, 256 * g:256 * (g + 1)].T).astype(bf16)
        in_maps.append({"xt": xt[b], "wqk": wqk, "wv": wv, "wo": wo})

    nc = _get_nc()
    res = run_bass_kernel_spmd(nc, in_maps, core_ids=list(range(N_CORES)))

    out = np.empty((B, T, D), dtype=np.float32)
    for i in range(N_CORES):
        b, r = divmod(i, 4)
        o = res.results[i]["out"].astype(np.float32)
        for qb in range(N_QB):
            out[b, QT * qb + 128 * r:QT * qb + 128 * (r + 1), :] = \
                o[128 * qb:128 * (qb + 1)]
    return out
